# revision 27
# baseline (speedup 1.0000x reference)
"""Trainium2 Bass kernel for CombinedVectorField (CFG vector field + exact
Jacobian-trace divergence).

Math: with u = tanh(x@W1x + h@W1h + b1'), b1' = b1 + t*W1[256],
  v(x,h)  = u @ W2 + b2
  div(x,h)= sum_k (1-u_k^2) c_k = d0 - (u*u) @ c,   c_k = sum_i W1x[i,k] W2[k,i]
Output = concat[(1-gs)*v_null + gs*v_h, (1-gs)*div_null + gs*div_h].

Sharding: pure data parallel - each of the 8 cores takes 512 batch rows
(both guidance branches), weights replicated. All tensors feature-major
(transposed) on device so every matmul contracts over the partition dim.

Schedule: inputs stream over three DMA queues (sync/scalar HWDGE + gpsimd
SWDGE) in consumption order, split fine enough that the first z1 matmuls
start on the first ~96KB. Both guidance branches of a hidden chunk share
one 2-bank PSUM tile ([128,1024] f32) so a single tanh ACTIVATE covers
both branches; the last chunk is branch-split so the final
tanh->u^2->divergence chain is half as long. v-bias is fused into the
vector-engine PSUM->SBUF copy (bf16 out), the divergence bias into the
scalar-engine copy.

PSUM accumulation within a shared bank uses start=True only on the first
matmul that touches the bank (the start flag clears the whole bank's
has_written bits, so a later start=True would corrupt sibling column
regions; with bits cleared once, later matmuls overwrite-or-accumulate
per element correctly in any order).
"""
import sys

sys.path.insert(0, "/opt/trn_rl_repo")

import ml_dtypes
import numpy as np

import concourse.bass as bass
import concourse.tile as tile
from concourse import bacc, mybir
from concourse.bass_utils import run_bass_kernel_spmd
from concourse.vector_clock import ScopedClock


class _TrimTileContext(tile.TileContext):
    """TileContext with the final all-engine barrier dropped from the
    teardown and the mid barrier reduced to sem-only (no per-engine
    drains). The head drain still waits for every semaphore (incl.
    output-DMA completion) and semaphores are still cleared for the next
    execution; only the trailing barrier (nothing executes after it) is
    elided."""

    def _drain_and_barrier(self, tick_clock, wait_clock):
        drain_inst = self.nc.sync.drain()
        wait_clock.add_sem_waits(
            drain_inst.ins, ScopedClock({None: tick_clock.global_clock})
        )
        self.nc.all_engine_barrier(sem_only=True)
        popped = self.nc._tile_sem_poison_stack.pop()
        assert popped is self._sem_poison
        self.nc.clear_and_free_semaphores(list(self.sems.allocated().values()))


class _FastBacc(bacc.Bacc):
    """Bacc whose constructor-time all-engine barrier (after the const-tile
    memsets) is sem-only - the per-engine drains there cost ~1us of kernel
    head time and order nothing we rely on beyond the memsets, which the
    event-semaphore barrier already orders."""

    def all_engine_barrier(self, *, sem_only: bool = False):
        super().all_engine_barrier(sem_only=True)

F32 = mybir.dt.float32
BF16 = mybir.dt.bfloat16
AF = mybir.ActivationFunctionType
ALU = mybir.AluOpType

N_CORES = 8
B = 4096
DIM_X = 128
DIM_H = 128
HIDDEN = 512
R = B // N_CORES          # rows per core
HR = R // 2
NCH = HIDDEN // 128       # hidden chunks
W2W = NCH * DIM_X + NCH   # w2 chunks + cmat columns per branch
N_PREWARM = 7

_NC_CACHE = None


def _build():
    nc = _FastBacc("TRN2", target_bir_lowering=False, debug=False,
                   enable_asserts=False, monotonic_sem_count=0)

    # Input blobs in consumption order (branch-0 units run first, so hnT is
    # only needed once branch-1 starts). First 8 emitted DMAs get dedicated
    # completion-sem lanes; the late-needed WC23/SN are emitted last so they
    # take the reused lanes (waiting on the earliest-finishing DMAs).
    #   sync   : S1=[w1x_c0|xT_p1] S2=[xT_rest] WC1=[w1_c1] WC23=[w1_c23] SN
    #   scalar : A1=[w1h_c0|hT_p1]  A2=[hT_rest]  GG=[gs*W2]
    #   gpsimd : AUX  G1=[hnT]
    inS1 = nc.dram_tensor("inS1", [128, 256], BF16, kind="ExternalInput")
    inS2 = nc.dram_tensor("inS2", [128, R - 128], BF16, kind="ExternalInput")
    inWC1 = nc.dram_tensor("inWC1", [128, 256], BF16, kind="ExternalInput")
    inWC23 = nc.dram_tensor("inWC23", [128, 512], BF16, kind="ExternalInput")
    inSN = nc.dram_tensor("inSN", [128, W2W], BF16, kind="ExternalInput")
    inA1 = nc.dram_tensor("inA1", [128, 256], BF16, kind="ExternalInput")
    inA2 = nc.dram_tensor("inA2", [128, R - 128], BF16, kind="ExternalInput")
    inGG = nc.dram_tensor("inGG", [128, W2W], BF16, kind="ExternalInput")
    inG1 = nc.dram_tensor("inG1", [128, R], BF16, kind="ExternalInput")
    # aux cols: 0-3 b1' chunks, 4 b2, 5 d0
    aux = nc.dram_tensor("aux", [128, 6], F32, kind="ExternalInput")

    VO = nc.dram_tensor("VO", [DIM_X, R], BF16, kind="ExternalOutput")
    DO = nc.dram_tensor("DO", [1, R], F32, kind="ExternalOutput")

    with _TrimTileContext(nc) as tc:
        with tc.tile_pool(name="cst", bufs=1) as cst, \
             tc.tile_pool(name="act", bufs=8) as actp, \
             tc.tile_pool(name="out", bufs=1) as outp, \
             tc.tile_pool(name="psa", bufs=6, space="PSUM") as psa, \
             tc.tile_pool(name="psv", bufs=1, space="PSUM") as psv:
            # PE prewarm: dummy bf16 matmuls on a zeroed tile. The PE-HAM
            # controller grants full clock (2.4 GHz) only after ~3us of
            # CONTINUOUS PE activity, so this must bridge the input-DMA wait
            # without gaps - too few prewarms delays the grant by 5+ us.
            wrm = cst.tile([128, 512], BF16)
            nc.gpsimd.memset(wrm[:], 0.0)
            pwarm = psa.tile([128, 512], F32, tag="a")
            for _ in range(N_PREWARM):
                nc.tensor.matmul(pwarm[:], wrm[:, 0:128], wrm[:],
                                 start=True, stop=True, skip_group_check=True)

            # SBUF destination tiles; xT/hT/hnT each land via two DMAs into
            # column slices of one tile (subtile deps let the lo-half matmuls
            # start before the hi half arrives).
            s1t = cst.tile([128, 128 + R], BF16)
            nc.sync.dma_start(out=s1t[:, 0:256], in_=inS1[:])
            a1t = cst.tile([128, 128 + R], BF16)
            nc.scalar.dma_start(out=a1t[:, 0:256], in_=inA1[:])
            auxt = cst.tile([128, 6], F32)
            nc.gpsimd.dma_start(out=auxt[:], in_=aux[:])
            nc.sync.dma_start(out=s1t[:, 256:128 + R], in_=inS2[:])
            nc.scalar.dma_start(out=a1t[:, 256:128 + R], in_=inA2[:])
            g1t = cst.tile([128, R], BF16)
            nc.gpsimd.dma_start(out=g1t[:], in_=inG1[:])
            wc1t = cst.tile([128, 256], BF16)
            nc.sync.dma_start(out=wc1t[:], in_=inWC1[:])
            gGt = cst.tile([128, W2W], BF16)
            nc.scalar.dma_start(out=gGt[:], in_=inGG[:])
            wc23t = cst.tile([128, 512], BF16)
            nc.sync.dma_start(out=wc23t[:], in_=inWC23[:])
            sNt = cst.tile([128, W2W], BF16)
            nc.sync.dma_start(out=sNt[:], in_=inSN[:])

            xt = s1t[:, 128:128 + R]
            hst = [a1t[:, 128:128 + R], g1t[:]]          # hT, hnT
            w1x = [s1t[:, 0:128], wc1t[:, 0:128],
                   wc23t[:, 0:128], wc23t[:, 256:384]]
            w1h = [a1t[:, 0:128], wc1t[:, 128:256],
                   wc23t[:, 128:256], wc23t[:, 384:512]]
            # branch 0 = gs-scaled (gpsimd blob), branch 1 = (1-gs)-scaled
            w2b = [gGt, sNt]

            # Units in branch-0-first order: all of branch 0's z1/tanh
            # proceed without hnT; branch 1 follows once hnT lands. One PSUM
            # bank per unit so each tanh waits only on its own two matmuls
            # (PSUM tiles have tile-granular deps). The first unit is
            # row-piecewise so matmuls start on the first 128-row DMA piece.
            units = [(c, 0) for c in range(NCH)] + [(c, 1) for c in range(NCH)]
            ats = {}
            for c, br in units:
                a = psa.tile([128, R], F32, tag="a")
                ats[(c, br)] = a
                if c == 0 and br == 0:
                    for i, (r0, r1) in enumerate([(0, 128), (128, 512)]):
                        nc.tensor.matmul(a[:, r0:r1], w1x[0], xt[:, r0:r1],
                                         start=(i == 0), stop=False,
                                         skip_group_check=True)
                        nc.tensor.matmul(a[:, r0:r1], w1h[0], hst[0][:, r0:r1],
                                         start=False, stop=(i == 1),
                                         skip_group_check=True)
                else:
                    nc.tensor.matmul(a[:], w1x[c], xt, start=True, stop=False)
                    nc.tensor.matmul(a[:], w1h[c], hst[br][:],
                                     start=False, stop=True)

            # tanh + u^2 per unit, same order
            us, u2s = {}, {}
            for c, br in units:
                u = actp.tile([128, R], BF16, tag="u")
                nc.scalar.activation(u[:], ats[(c, br)][:], AF.Tanh,
                                     bias=auxt[:, c:c + 1], scale=1.0)
                us[(c, br)] = u
                u2 = actp.tile([128, R], BF16, tag="u2")
                nc.vector.tensor_tensor(u2[:], u[:], u[:], op=ALU.mult)
                u2s[(c, br)] = u2

            # weights pre-scaled by gs/(1-gs)/-gs/-(1-gs): the PSUM sums ARE
            # the guidance-combined results. pv emitted before pd per unit so
            # the big VO output starts as early as possible.
            pv = psv.tile([128, R], F32)
            pd = psv.tile([1, R], F32)
            for i, (c, br) in enumerate(units):
                wc = slice(c * 128, (c + 1) * 128)
                cc = slice(NCH * DIM_X + c, NCH * DIM_X + c + 1)
                first, last = i == 0, i == len(units) - 1
                nc.tensor.matmul(pv[:], w2b[br][:, wc], us[(c, br)][:],
                                 start=first, stop=last)
                nc.tensor.matmul(pd[0:1, :], w2b[br][:, cc], u2s[(c, br)][:],
                                 start=first, stop=last)

            # v-bias fused into the vector copy (bf16 out); div-bias into the
            # scalar copy - the two PSUM->SBUF moves run on different engines.
            vout = outp.tile([128, R], BF16)
            nc.vector.tensor_scalar(vout[:], pv[:], auxt[:, 4:5], None, op0=ALU.add)
            dout = outp.tile([1, R], F32)
            nc.scalar.activation(dout[:], pd[0:1, :], AF.Identity,
                                 bias=auxt[0:1, 5:6], scale=1.0)

            nc.sync.dma_start(out=VO[:], in_=vout[:])
            nc.scalar.dma_start(out=DO[:], in_=dout[:])
    nc.compile()
    return nc


def _get_nc():
    global _NC_CACHE
    if _NC_CACHE is None:
        _NC_CACHE = _build()
    return _NC_CACHE


def _prep_in_maps(state, h, h_null, t, guidance_scale, W1, b1, W2, b2):
    f32 = np.float32
    bf = ml_dtypes.bfloat16
    xTf = state[:, :DIM_X].T.astype(bf)                            # (128, B)
    hTf = h.T.astype(bf)
    hnTf = h_null.T.astype(bf)
    w1xf = W1[:DIM_X].astype(bf)                                   # (128, 512)
    w1hf = W1[DIM_X:DIM_X + DIM_H].astype(bf)
    b1p = (b1.astype(f32) + t.astype(f32)[0] * W1[DIM_X + DIM_H].astype(f32))
    w2r = W2.astype(f32).reshape(NCH, 128, DIM_X).transpose(1, 0, 2).reshape(128, NCH * DIM_X)
    cvec = (W1[:DIM_X].astype(np.float64) * W2.astype(np.float64).T).sum(0)  # (512,)
    d0 = cvec.sum()
    cmatf = cvec.reshape(NCH, 128).T.astype(f32)                   # (128, NCH)
    gs = float(guidance_scale.astype(f32)[0])
    blob_gs = np.concatenate([gs * w2r, -gs * cmatf], axis=1).astype(bf)
    blob_n = np.concatenate([(1.0 - gs) * w2r, -(1.0 - gs) * cmatf], axis=1).astype(bf)

    auxf = np.zeros((128, 6), f32)
    auxf[:, 0:4] = b1p.reshape(NCH, 128).T
    auxf[:, 4] = b2.astype(f32)
    auxf[:, 5] = d0

    wc1 = np.ascontiguousarray(
        np.concatenate([w1xf[:, 128:256], w1hf[:, 128:256]], axis=1))
    wc23 = np.ascontiguousarray(
        np.concatenate([w1xf[:, 256:384], w1hf[:, 256:384],
                        w1xf[:, 384:512], w1hf[:, 384:512]], axis=1))
    in_maps = []
    for i in range(N_CORES):
        o = i * R
        in_maps.append({
            "inS1": np.ascontiguousarray(
                np.concatenate([w1xf[:, 0:128], xTf[:, o:o + 128]], axis=1)),
            "inS2": np.ascontiguousarray(xTf[:, o + 128:o + R]),
            "inWC1": wc1,
            "inWC23": wc23,
            "inSN": blob_n,
            "inA1": np.ascontiguousarray(
                np.concatenate([w1hf[:, 0:128], hTf[:, o:o + 128]], axis=1)),
            "inA2": np.ascontiguousarray(hTf[:, o + 128:o + R]),
            "inGG": blob_gs,
            "inG1": np.ascontiguousarray(hnTf[:, o:o + R]),
            "aux": auxf,
        })
    return in_maps


def kernel(state, h, h_null, t, guidance_scale, W1, b1, W2, b2, _trace=False):
    nc = _get_nc()
    in_maps = _prep_in_maps(state, h, h_null, t, guidance_scale, W1, b1, W2, b2)
    res = run_bass_kernel_spmd(nc, in_maps, list(range(N_CORES)), trace=_trace)
    out = np.empty((B, DIM_X + 1), np.float32)
    for i in range(N_CORES):
        sl = slice(i * R, (i + 1) * R)
        out[sl, :DIM_X] = res.results[i]["VO"].astype(np.float32).T
        out[sl, DIM_X] = res.results[i]["DO"][0]
    if _trace:
        return out, res
    return out
